# revision 5
# baseline (speedup 1.0000x reference)
"""Trainium2 Bass kernel for nn_C2f_DualModal_MoE (C2f block with top-1 MoE routing).

Strategy (data-parallel over batch, 4 samples per core on 8 cores):
  - cv1 (1x1 conv 256->256 + SiLU) in bf16 as matmuls over 400-pixel tiles;
    `feat` goes into a zero-padded [82x82] spatial layout (bf16 for cv2, plus
    an fp8e4m3 copy for the 3x3 convs). Router GAP comes free via ACT
    accum_out.
  - The two 3x3 convs (shared + selected routed expert) run in fp8 with
    DoubleRow perf mode: taps are paired into 4 DoubleRow matmuls (2 k-tiles
    of 128 each) + 1 plain fp8 matmul, over flat 410-column tiles (5 padded
    rows of 82; ~2.5% wasted columns), writing a row-padded 80x82 output
    layout. Weights are pre-scaled by 2^8 to sit in e4m3's normal range; the
    SiLU activation un-scales via its scale operand.
  - Router: tiny matmul + softmax on-chip; top-1 turned into a one-hot that
    selects the routed expert's weights via vector ops (then converted to
    fp8). Router PE ops are interleaved between shared-conv tiles so the PE
    never waits on the vector-engine softmax chain.
  - cv2 (1x1 conv 384->256 + SiLU) in bf16, fused per tile from (a, feat,
    moe) with no materialized concat; routed-conv and cv2 are
    software-pipelined by one tile-pair.
  - x loads ride the SP DMA queue; y stores ride the ACT DMA queue so
    next-sample prefetch never queues behind output drains.
"""

import numpy as np

import concourse.bass as bass
import concourse.bacc as bacc
import concourse.tile as tile
from concourse import mybir
from concourse.bass_utils import run_bass_kernel_spmd

# Problem constants (hardcoded per contract)
B, C1, C2 = 32, 256, 256
H = W = 80
CH = 128
NE = 3
NCORES = 8
BPC = B // NCORES          # samples per core = 4
NPIX = H * W               # 6400
PADW = W + 2               # 82
PADH = H + 2               # 82
PADN = PADH * PADW         # 6724
FLATN = H * PADW           # 6560 (output positions in row-padded layout)
RPT = 5                    # rows per tile
TN = RPT * W               # 400 px per cv1/cv2 tile
FN = RPT * PADW            # 410 px per conv tile (flat, incl 2 pad cols/row)
NT = H // RPT              # 16 tiles
NP = NT // 2               # 8 tile-pairs
WSCALE = 256.0             # fp8 conv-weight pre-scale (undone in ACT)

# conv tap pairing for DoubleRow: weight groups are laid out host-side in
# this tap order; rhs offsets are in the flat padded [82x82] layout.
TAP_PERM = [0, 1, 3, 4, 6, 7, 2, 5, 8]        # (dy*3+dx) order
DR_PAIRS = [(0, 1), (82, 1), (164, 1), (2, 82)]  # (start_off, j_stride)
SINGLE_OFF = 166                               # tap (2,2)

f32 = mybir.dt.float32
f32r = mybir.dt.float32r
bf16 = mybir.dt.bfloat16
fp8 = mybir.dt.float8e4


def _sview(base, off, dims):
    """Custom (possibly overlapping) strided free-dim view of a 2D SBUF AP."""
    return bass.AP(
        tensor=base.tensor,
        offset=base.offset + off,
        ap=[list(base.ap[0])] + [list(d) for d in dims],
        const_val=base.const_val,
        runtime_checks=base.runtime_checks,
        dep_tracking_offset=base.dep_tracking_offset,
    )


def _emit(nc, tc, ctx, reps=1, sim_compat=False, tune=None, internal_io=False):
    AX = mybir.AxisListType
    OP = mybir.AluOpType
    AF = mybir.ActivationFunctionType
    DR = mybir.MatmulPerfMode.DoubleRow
    tune = {**dict(xbufs=4, obufs=4, rbufs=2, psbufs=3, ybf16=True),
            **(tune or {})}

    io_kind = "Internal" if internal_io else "ExternalInput"
    ydt = bf16 if tune["ybf16"] else f32
    x_d = nc.dram_tensor("x", [BPC, 2, CH, NPIX], bf16, kind=io_kind).ap()
    w1_d = nc.dram_tensor("w1t", [2, CH, 2 * CH], bf16, kind="ExternalInput").ap()
    b1_d = nc.dram_tensor("b1r", [2, CH], f32, kind="ExternalInput").ap()
    wr_d = nc.dram_tensor("wrs", [CH, NE], f32, kind="ExternalInput").ap()
    br_d = nc.dram_tensor("brr", [1, NE], f32, kind="ExternalInput").ap()
    ws_d = nc.dram_tensor("ws8", [CH, 9 * CH], fp8, kind="ExternalInput").ap()
    bs_d = nc.dram_tensor("bsr", [CH, 1], f32, kind="ExternalInput").ap()
    we_d = nc.dram_tensor("wet", [NE, CH, 9 * CH], f32, kind="ExternalInput").ap()
    be_d = nc.dram_tensor("ber", [CH, NE], f32, kind="ExternalInput").ap()
    w2_d = nc.dram_tensor("w2t", [3, CH, C2], bf16, kind="ExternalInput").ap()
    b2_d = nc.dram_tensor("b2r", [2, CH], f32, kind="ExternalInput").ap()
    y_d = nc.dram_tensor(
        "y", [BPC, 2, CH, NPIX], ydt,
        kind="Internal" if internal_io else "ExternalOutput").ap()

    wpool = ctx.enter_context(tc.tile_pool(name="weights", bufs=1))
    ppool = ctx.enter_context(tc.tile_pool(name="persist", bufs=1))
    xpool = ctx.enter_context(tc.tile_pool(name="xin", bufs=tune["xbufs"]))
    opool = ctx.enter_context(tc.tile_pool(name="oout", bufs=tune["obufs"]))
    rpool = ctx.enter_context(tc.tile_pool(name="rtile", bufs=tune["rbufs"]))
    spool = ctx.enter_context(tc.tile_pool(name="small", bufs=2))
    selpool = ctx.enter_context(tc.tile_pool(name="sel", bufs=1))
    psum = ctx.enter_context(tc.tile_pool(name="psum", bufs=tune["psbufs"], space="PSUM"))
    psumS = ctx.enter_context(tc.tile_pool(name="psumS", bufs=1, space="PSUM"))

    # ---- load weights into SBUF (resident) ----
    w1_sb = wpool.tile([CH, 2 * 2 * CH], bf16)
    for k in range(2):
        nc.sync.dma_start(w1_sb[:, k * 256:(k + 1) * 256], w1_d[k])
    ws8_sb = wpool.tile([CH, 9 * CH], fp8)
    nc.sync.dma_start(ws8_sb[:], ws_d)
    we_sb = wpool.tile([CH, NE * 9 * CH], f32)
    for e in range(NE):
        nc.sync.dma_start(we_sb[:, e * 1152:(e + 1) * 1152], we_d[e])
    w2_sb = wpool.tile([CH, 3 * C2], bf16)
    for k in range(3):
        nc.sync.dma_start(w2_sb[:, k * 256:(k + 1) * 256], w2_d[k])
    wr_sb = wpool.tile([CH, NE], f32)
    nc.sync.dma_start(wr_sb[:], wr_d)
    br_sb = wpool.tile([1, NE], f32)
    nc.sync.dma_start(br_sb[:], br_d)
    bs_sb = wpool.tile([CH, 1], f32)
    nc.sync.dma_start(bs_sb[:], bs_d)
    be_sb = wpool.tile([CH, NE], f32)
    nc.sync.dma_start(be_sb[:], be_d)
    b1_sb = wpool.tile([CH, 2], f32)
    for k in range(2):
        nc.sync.dma_start(b1_sb[:, k:k + 1], b1_d[k])
    b2_sb = wpool.tile([CH, 2], f32)
    for k in range(2):
        nc.sync.dma_start(b2_sb[:, k:k + 1], b2_d[k])
    ones_sb = wpool.tile([1, CH], f32)
    nc.vector.memset(ones_sb[:], 1.0)

    if internal_io:
        # timing mode: x is Internal (uninitialized) DRAM; zero it once so
        # the timed loop computes on deterministic, non-denormal data.
        zs = wpool.tile([CH, 800], bf16, name="zs")
        nc.vector.memset(zs[:], 0.0)
        for zb in range(BPC):
            for zk in range(2):
                for zj in range(NPIX // 800):
                    nc.sync.dma_start(
                        x_d[zb, zk, :, zj * 800:(zj + 1) * 800], zs[:])

    # ---- persistent per-sample working buffers ----
    # fp (padded feat, bf16) and a are double-buffered across samples so the
    # next sample's cv1 writes don't wait on this sample's cv2 reads.
    fps = []
    for fi in range(2):
        fp = ppool.tile([CH, PADN], bf16, tag=f"fp{fi}", name=f"fp{fi}")
        nc.vector.memset(fp[:], 0.0)   # borders stay zero forever
        fps.append(fp)
    # +4 pad: the discarded (row-wrap) columns of the last conv tile read up
    # to 2 elements past the 82x82 image; keep those reads in-bounds and zero.
    fp8p = ppool.tile([CH, PADN + 4], fp8, name="fp8p")
    nc.vector.memset(fp8p[:].bitcast(f32), 0.0)
    a_sbs = [ppool.tile([CH, NPIX], bf16, tag=f"a{ai}", name=f"a{ai}")
             for ai in range(2)]
    sh_sb = ppool.tile([CH, FLATN], bf16)
    moe_sb = ppool.tile([CH, FLATN], bf16)

    tmpool = ctx.enter_context(tc.tile_pool(name="silutmp", bufs=2)) if sim_compat else None

    def act_silu(out_ap, ps_ap, bias_ap, scale=1.0, accum_ap=None):
        """SiLU from PSUM -> SBUF. On HW, one ACT instruction (with optional
        free GAP accumulation). CoreSim lacks Silu, so sim_compat emulates via
        Sigmoid + (ps*scale+bias)*sig, and computes the accumulation separately."""
        if not sim_compat:
            if accum_ap is not None:
                nc.scalar.activation(out_ap, ps_ap, AF.Silu, bias=bias_ap,
                                     scale=scale, accum_out=accum_ap)
            else:
                nc.scalar.activation(out_ap, ps_ap, AF.Silu, bias=bias_ap,
                                     scale=scale)
            return
        shp = list(out_ap.shape[1:])
        fs = 1
        for d in shp:
            fs *= d
        tmp = tmpool.tile([CH, 2 * 512], f32, tag="sigmoid_tmp")
        tmp2 = tmpool.tile([CH, 2 * 512], f32, tag="scaled_tmp")
        tv = tmp[:, 0:fs]
        sv = tmp2[:, 0:fs]
        if len(shp) == 2:
            tv = tv.rearrange("p (g c) -> p g c", g=shp[0])
            sv = sv.rearrange("p (g c) -> p g c", g=shp[0])
        elif len(shp) == 3:
            tv = tv.rearrange("p (g r c) -> p g r c", g=shp[0], r=shp[1])
            sv = sv.rearrange("p (g r c) -> p g r c", g=shp[0], r=shp[1])
        nc.scalar.activation(tv, ps_ap, AF.Sigmoid, bias=bias_ap, scale=scale)
        # sv = ps*scale + bias ; out = sv * sigmoid(sv)
        nc.vector.tensor_scalar(sv, ps_ap, scale, bias_ap, op0=OP.mult, op1=OP.add)
        nc.vector.tensor_mul(out_ap, sv, tv)
        if accum_ap is not None:
            axis = [None, AX.X, AX.XY, AX.XYZ][len(shp)]
            nc.vector.reduce_sum(accum_ap, out_ap, axis=axis)

    def conv_tile(ps_view, w8, fp8base, t):
        """3x3 conv over flat tile t (410 cols) in fp8: 4 DoubleRow + 1."""
        base = t * RPT * PADW
        for j, (soff, jstr) in enumerate(DR_PAIRS):
            lhs = w8[:, j * 256:(j + 1) * 256].rearrange(
                "p (j c) -> p j c", j=2)
            rhs = _sview(fp8base, base + soff, [[jstr, 2], [1, FN]])
            nc.tensor.matmul(ps_view, lhs, rhs, start=(j == 0), stop=False,
                             perf_mode=DR)
        rhs = _sview(fp8base, base + SINGLE_OFF, [[1, FN]])
        nc.tensor.matmul(ps_view, w8[:, 8 * CH:9 * CH], rhs,
                         start=False, stop=True)

    def _body():
        for b in range(BPC):
            fp = fps[b % 2]
            fp3 = fp[:].rearrange("p (r c) -> p r c", c=PADW)
            a_sb = a_sbs[b % 2]
            fp8base = fp8p[:]
            # ---- cv1 over tile-PAIRS: 800 px per ACT, GAP accumulated free;
            # DVE mirrors feat into the fp8 padded buffer ----
            gap_sb = spool.tile([CH, NP], f32, tag="gap")
            for pi in range(NP):
                i0 = 2 * pi
                xt0 = xpool.tile([CH, 2 * TN], bf16, tag="xt0")
                nc.sync.dma_start(xt0[:], x_d[b, 0, :, pi * 800:(pi + 1) * 800])
                xt1 = xpool.tile([CH, 2 * TN], bf16, tag="xt1")
                nc.sync.dma_start(xt1[:], x_d[b, 1, :, pi * 800:(pi + 1) * 800])
                ps_a = psum.tile([CH, 2, 512], f32, tag="ps")
                ps_f = psum.tile([CH, 2, 512], f32, tag="ps")
                for k, xt in ((0, xt0), (1, xt1)):
                    for hw_, ps2 in ((0, ps_a), (1, ps_f)):
                        wsl = w1_sb[:, k * 256 + hw_ * 128: k * 256 + hw_ * 128 + 128]
                        for ii in range(2):
                            nc.tensor.matmul(ps2[:, ii, 0:TN], wsl,
                                             xt[:, ii * TN:(ii + 1) * TN],
                                             start=(k == 0), stop=(k == 1))
                act_silu(a_sb[:, i0 * TN:(i0 + 2) * TN].rearrange(
                             "p (g c) -> p g c", g=2),
                         ps_a[:, :, 0:TN], b1_sb[:, 0:1])
                fout = fp3[:, 1 + 10 * pi: 11 + 10 * pi, 1:1 + W].rearrange(
                    "p (g r) c -> p g r c", g=2)
                act_silu(fout,
                         ps_f[:, :, 0:TN].rearrange("p g (r c) -> p g r c", c=W),
                         b1_sb[:, 1:2], accum_ap=gap_sb[:, pi:pi + 1])
                # fp8 mirror of this pair's feat rows (for the 3x3 convs)
                nc.vector.tensor_copy(
                    _sview(fp8base, (1 + 10 * pi) * PADW + 1, [[PADW, 10], [1, W]]),
                    fp3[:, 1 + 10 * pi: 11 + 10 * pi, 1:1 + W])

            # ---- router part 1: pooled sum (wr pre-scaled by 1/NPIX) ----
            pooled = spool.tile([CH, 1], f32, tag="pooled")
            nc.vector.reduce_sum(pooled[:], gap_sb[:], axis=AX.X)

            # ---- shared expert 3x3 conv + SiLU (fp8 DoubleRow), with the
            # router's two tiny PE ops interleaved after tiles 0 and 2 so the
            # PE never stalls on the DVE softmax chain ----
            ps_l = psumS.tile([1, NE], f32, tag="psl")
            ps_bc = psumS.tile([CH, NE + 1], f32, tag="psb")

            def emit_router_mm():
                nc.tensor.matmul(ps_l[:], pooled[:], wr_sb[:], start=True, stop=True)

            def emit_softmax_dve():
                logits = spool.tile([1, NE], f32, tag="logits")
                nc.vector.tensor_add(logits[:], ps_l[:], br_sb[:])
                m_sb = spool.tile([1, 1], f32, tag="m")
                nc.vector.reduce_max(m_sb[:], logits[:], axis=AX.X)
                negm = spool.tile([1, 1], f32, tag="negm")
                nc.vector.tensor_scalar_mul(negm[:], m_sb[:], -1.0)
                e_sb = spool.tile([1, NE], f32, tag="esb")
                nc.scalar.activation(e_sb[:], logits[:], AF.Exp, bias=negm[:], scale=1.0)
                s_sb = spool.tile([1, 1], f32, tag="ssb")
                nc.vector.reduce_sum(s_sb[:], e_sb[:], axis=AX.X)
                wgt = spool.tile([1, 1], f32, tag="wgt")
                nc.vector.reciprocal(wgt[:], s_sb[:])
                oh = spool.tile([1, NE], f32, tag="oh")
                nc.vector.tensor_scalar(oh[:], logits[:], m_sb[:], None, op0=OP.is_ge)
                bc = spool.tile([1, NE + 1], f32, tag="bc")
                nc.vector.tensor_copy(bc[:, 0:NE], oh[:])
                nc.vector.tensor_copy(bc[:, NE:NE + 1], wgt[:])
                return bc

            def emit_bcast_mm(bc):
                nc.tensor.matmul(ps_bc[:], ones_sb[:], bc[:], start=True, stop=True)

            def emit_select():
                sc = spool.tile([CH, NE + 1], f32, tag="sc")
                nc.vector.tensor_copy(sc[:], ps_bc[:])
                bA = spool.tile([CH, 1], f32, tag="bA")
                nc.vector.tensor_scalar_mul(bA[:], be_sb[:, 0:1], sc[:, 0:1])
                bB = spool.tile([CH, 1], f32, tag="bB")
                nc.vector.scalar_tensor_tensor(bB[:], be_sb[:, 1:2], sc[:, 1:2],
                                               bA[:], op0=OP.mult, op1=OP.add)
                bS = spool.tile([CH, 1], f32, tag="bS")
                nc.vector.scalar_tensor_tensor(bS[:], be_sb[:, 2:3], sc[:, 2:3],
                                               bB[:], op0=OP.mult, op1=OP.add)
                wA = selpool.tile([CH, 9 * CH], f32, tag="wA")
                nc.vector.tensor_scalar_mul(wA[:], we_sb[:, 0:1152], sc[:, 0:1])
                wB = selpool.tile([CH, 9 * CH], f32, tag="wB")
                nc.vector.scalar_tensor_tensor(wB[:], we_sb[:, 1152:2304], sc[:, 1:2],
                                               wA[:], op0=OP.mult, op1=OP.add)
                wC = selpool.tile([CH, 9 * CH], f32, tag="wC")
                nc.vector.scalar_tensor_tensor(wC[:], we_sb[:, 2304:3456], sc[:, 2:3],
                                               wB[:], op0=OP.mult, op1=OP.add)
                wS8 = selpool.tile([CH, 9 * CH], fp8, tag="wS8")
                nc.vector.tensor_copy(wS8[:], wC[:])
                return wS8, bS, sc

            bc = None
            for pi in range(NP):
                ps2 = psum.tile([CH, 2, 512], f32, tag="ps")
                for ii in range(2):
                    conv_tile(ps2[:, ii, 0:FN], ws8_sb[:], fp8base, 2 * pi + ii)
                act_silu(sh_sb[:, pi * 2 * FN:(pi + 1) * 2 * FN].rearrange(
                             "p (g c) -> p g c", g=2),
                         ps2[:, :, 0:FN], bs_sb[:], scale=1.0 / WSCALE)
                if pi == 0:
                    emit_router_mm()
                    bc = emit_softmax_dve()
                elif pi == 1:
                    emit_bcast_mm(bc)
                elif pi == 2:
                    wS8, bS, sc = emit_select()

            # ---- routed conv (fp8 DR) + moe + fused cv2 (bf16),
            # software-pipelined by one pair ----
            def cv2_pair(pi):
                i0 = 2 * pi
                moe3 = moe_sb[:].rearrange("p (r c) -> p r c", c=PADW)
                for h in range(2):
                    po = psum.tile([CH, 2, 512], f32, tag="ps")
                    for ii in range(2):
                        i = i0 + ii
                        ft = fp3[:, i * RPT + 1: i * RPT + 1 + RPT, 1: 1 + W]
                        mt = moe3[:, i * RPT: i * RPT + RPT, 0:W]
                        nc.tensor.matmul(po[:, ii, 0:TN],
                                         w2_sb[:, h * 128: h * 128 + 128],
                                         a_sb[:, i * TN:(i + 1) * TN],
                                         start=True, stop=False)
                        nc.tensor.matmul(po[:, ii, 0:TN],
                                         w2_sb[:, 256 + h * 128: 256 + h * 128 + 128],
                                         ft, start=False, stop=False)
                        nc.tensor.matmul(po[:, ii, 0:TN],
                                         w2_sb[:, 512 + h * 128: 512 + h * 128 + 128],
                                         mt, start=False, stop=True)
                    ot = opool.tile([CH, 2 * TN], ydt, tag="ot")
                    act_silu(ot[:].rearrange("p (g c) -> p g c", g=2),
                             po[:, :, 0:TN], b2_sb[:, h:h + 1])
                    # y stores ride the ACT DMA queue (keeps SP free for x)
                    nc.scalar.dma_start(y_d[b, h, :, pi * 800:(pi + 1) * 800], ot[:])

            for pi in range(NP):
                ps2 = psum.tile([CH, 2, 512], f32, tag="ps")
                for ii in range(2):
                    conv_tile(ps2[:, ii, 0:FN], wS8[:], fp8base, 2 * pi + ii)
                rt = rpool.tile([CH, 2 * FN], bf16, tag="rt")
                act_silu(rt[:].rearrange("p (g c) -> p g c", g=2),
                         ps2[:, :, 0:FN], bS[:], scale=1.0 / WSCALE)
                nc.vector.scalar_tensor_tensor(
                    moe_sb[:, pi * 2 * FN:(pi + 1) * 2 * FN], rt[:], sc[:, NE:NE + 1],
                    sh_sb[:, pi * 2 * FN:(pi + 1) * 2 * FN], op0=OP.mult, op1=OP.add)
                if pi > 0:
                    cv2_pair(pi - 1)
            cv2_pair(NP - 1)

    if reps == 1:
        _body()
    else:
        # HW timing mode: repeat the whole workload in a hardware loop
        # (same instruction count / compile cost; R x device work).
        with tc.For_i(0, reps, 1):
            _body()
    if internal_io:
        # tiny external output so the (otherwise internal-IO) program is not
        # dead-code eliminated; depends on the looped work via y.
        ydig_d = nc.dram_tensor("ydig", [CH, 4], ydt,
                                kind="ExternalOutput").ap()
        ydig_t = opool.tile([CH, 4], ydt, name="ydig_t")
        nc.sync.dma_start(ydig_t[:], y_d[0, 0, :, 0:4])
        nc.sync.dma_start(ydig_d, ydig_t[:])


def build(reps=1, sim_compat=False, tune=None, internal_io=False):
    from contextlib import ExitStack
    nc = bacc.Bacc("TRN2", target_bir_lowering=False, debug=False,
                   num_devices=NCORES)
    with tile.TileContext(nc) as tc:
        with ExitStack() as ctx:
            _emit(nc, tc, ctx, reps=reps, sim_compat=sim_compat, tune=tune,
                  internal_io=internal_io)
    nc.compile()
    return nc


def marshal_inputs(x, w1, b1, wr, br, ws, bs, we, be, w2, b2, ybf16=True):
    """Host-side (tiny) weight re-layouts into matmul-friendly forms."""
    import ml_dtypes
    asf = lambda a: np.ascontiguousarray(np.asarray(a, dtype=np.float32))
    cvtb = lambda a: np.ascontiguousarray(
        np.asarray(a, np.float32).astype(ml_dtypes.bfloat16))
    cvt8 = lambda a: np.ascontiguousarray(
        np.asarray(a, np.float32).astype(ml_dtypes.float8_e4m3))
    x = cvtb(x)
    w1t = cvtb(np.asarray(w1, np.float32).reshape(2 * CH, C1).T.reshape(2, CH, 2 * CH))
    b1r = asf(np.asarray(b1, np.float32).reshape(2, CH))
    wrs = asf(np.asarray(wr, np.float32) / NPIX)
    brr = asf(np.asarray(br, np.float32).reshape(1, NE))
    # conv weights: [cin, tap, cout] with taps permuted for DoubleRow pairing,
    # pre-scaled by 2^8 into e4m3's normal range.
    wst = np.asarray(ws, np.float32).transpose(1, 2, 3, 0).reshape(CH, 9, CH)
    ws8 = cvt8(wst[:, TAP_PERM, :].reshape(CH, 9 * CH) * WSCALE)
    bsr = asf(np.asarray(bs, np.float32).reshape(CH, 1))
    wet_ = np.asarray(we, np.float32).transpose(0, 2, 3, 4, 1).reshape(NE, CH, 9, CH)
    # quantize to the fp8 grid on host, ship as f32 (exact on-device select)
    wet = asf(cvt8(wet_[:, :, TAP_PERM, :].reshape(NE, CH, 9 * CH) * WSCALE))
    ber = asf(np.asarray(be, np.float32).T)
    w2t = cvtb(np.asarray(w2, np.float32).reshape(C2, 3 * CH).T.reshape(3, CH, C2))
    b2r = asf(np.asarray(b2, np.float32).reshape(2, CH))
    shared = dict(w1t=w1t, b1r=b1r, wrs=wrs, brr=brr, ws8=ws8, bsr=bsr,
                  wet=wet, ber=ber, w2t=w2t, b2r=b2r)
    xc = x.reshape(NCORES, BPC, 2, CH, NPIX)
    in_maps = [dict(shared, x=np.ascontiguousarray(xc[c])) for c in range(NCORES)]
    return in_maps


_CACHE = {}


def _get_nc():
    if "nc" not in _CACHE:
        _CACHE["nc"] = build(reps=1)
    return _CACHE["nc"]


def _get_runner():
    """Build the sharded PJRT callable once (mirrors
    bass2jax.run_bass_via_pjrt's multi-core path) so repeat kernel() calls
    skip the jax retrace/compile."""
    if "runner" in _CACHE:
        return _CACHE["runner"]
    import jax
    from jax.experimental.shard_map import shard_map
    from jax.sharding import Mesh, PartitionSpec
    from concourse import bass2jax

    nc = _get_nc()
    bass2jax.install_neuronx_cc_hook()
    part_name = nc.partition_id_tensor.name if nc.partition_id_tensor else None
    in_names, out_names, out_avals = [], [], []
    for alloc in nc.m.functions[0].allocations:
        if not isinstance(alloc, mybir.MemoryLocationSet):
            continue
        name = alloc.memorylocations[0].name
        if alloc.kind == "ExternalInput":
            if name != part_name:
                in_names.append(name)
        elif alloc.kind == "ExternalOutput":
            out_names.append(name)
            out_avals.append(jax.core.ShapedArray(
                tuple(alloc.tensor_shape), mybir.dt.np(alloc.dtype)))
    assert nc.dbg_addr is None
    n_params = len(in_names)
    all_in = in_names + out_names  # zero buffers donated as outputs
    if part_name is not None:
        all_in = all_in + [part_name]

    def _body(*args):
        operands = list(args)
        if part_name is not None:
            operands.append(bass2jax.partition_id_tensor())
        outs = bass2jax._bass_exec_p.bind(
            *operands, out_avals=tuple(out_avals), in_names=tuple(all_in),
            out_names=tuple(out_names), lowering_input_output_aliases=(),
            sim_require_finite=True, sim_require_nnan=True, nc=nc)
        return tuple(outs)

    devices = jax.devices()[:NCORES]
    mesh = Mesh(np.asarray(devices), ("core",))
    nio = n_params + len(out_names)
    sharded = jax.jit(
        shard_map(_body, mesh=mesh, in_specs=(PartitionSpec("core"),) * nio,
                  out_specs=(PartitionSpec("core"),) * len(out_names),
                  check_rep=False),
        donate_argnums=tuple(range(n_params, nio)), keep_unused=True)
    _CACHE["runner"] = (sharded, in_names, out_names, out_avals)
    return _CACHE["runner"]


def kernel(x, w1, b1, wr, br, ws, bs, we, be, w2, b2):
    in_maps = marshal_inputs(x, w1, b1, wr, br, ws, bs, we, be, w2, b2)
    sharded, in_names, out_names, out_avals = _get_runner()
    concat_in = [
        np.concatenate([in_maps[c][name] for c in range(NCORES)], axis=0)
        for name in in_names
    ]
    concat_zeros = [
        np.zeros((NCORES * a.shape[0], *a.shape[1:]), a.dtype) for a in out_avals
    ]
    out_arrs = sharded(*concat_in, *concat_zeros)
    y = np.asarray(out_arrs[out_names.index("y")]).astype(np.float32)
    return np.ascontiguousarray(y.reshape(B, C2, H, W))


# revision 12
# speedup vs baseline: 1.0357x; 1.0357x over previous
"""Trainium2 Bass kernel for nn_C2f_DualModal_MoE (C2f block with top-1 MoE routing).

Strategy (data-parallel over batch, 4 samples per core on 8 cores):
  - cv1 (1x1 conv 256->256 + SiLU) in bf16 as matmuls over 400-pixel tiles;
    `feat` goes into a zero-padded [82x82] spatial layout (bf16 for cv2, plus
    an fp8e4m3 copy for the 3x3 convs). Router GAP comes free via ACT
    accum_out.
  - The two 3x3 convs (shared + selected routed expert) run in fp8 with
    DoubleRow perf mode: taps are paired into 4 DoubleRow matmuls (2 k-tiles
    of 128 each) + 1 plain fp8 matmul, over flat 410-column tiles (5 padded
    rows of 82; ~2.5% wasted columns), writing a row-padded 80x82 output
    layout. Weights are pre-scaled by 2^8 to sit in e4m3's normal range; the
    SiLU activation un-scales via its scale operand.
  - Router: tiny matmul + softmax on-chip; top-1 turned into a one-hot that
    selects the routed expert's weights via vector ops (then converted to
    fp8). Router PE ops are interleaved between shared-conv tiles so the PE
    never waits on the vector-engine softmax chain.
  - cv2 (1x1 conv 384->256 + SiLU) in bf16, fused per tile from (a, feat,
    moe) with no materialized concat; routed-conv and cv2 are
    software-pipelined by one tile-pair.
  - x loads ride the SP DMA queue; y stores ride the ACT DMA queue so
    next-sample prefetch never queues behind output drains.
"""

import numpy as np

import concourse.bass as bass
import concourse.bacc as bacc
import concourse.tile as tile
from concourse import mybir
from concourse.bass_utils import run_bass_kernel_spmd

# Problem constants (hardcoded per contract)
B, C1, C2 = 32, 256, 256
H = W = 80
CH = 128
NE = 3
NCORES = 8
BPC = B // NCORES          # samples per core = 4
NPIX = H * W               # 6400
PADW = W + 2               # 82
PADH = H + 2               # 82
PADN = PADH * PADW         # 6724
FLATN = H * PADW           # 6560 (output positions in row-padded layout)
RPT = 5                    # rows per tile
TN = RPT * W               # 400 px per cv1/cv2 tile
FN = RPT * PADW            # 410 px per conv tile (flat, incl 2 pad cols/row)
NT = H // RPT              # 16 tiles
NP = NT // 2               # 8 tile-pairs
WSCALE = 256.0             # fp8 conv-weight pre-scale (undone in ACT)

# conv tap pairing for DoubleRow: weight groups are laid out host-side in
# this tap order (plus a zero-weight 10th tap paired with tap 8, so all five
# groups ride DoubleRow); rhs offsets are in the flat padded [82x82] layout.
TAP_PERM = [0, 1, 3, 4, 6, 7, 2, 5, 8]        # (dy*3+dx) order
NTAP = 10                                      # 9 real + 1 zero pad
DR_PAIRS = [(0, 1), (82, 1), (164, 1), (2, 82), (166, 1)]  # (start_off, j_stride)

f32 = mybir.dt.float32
f32r = mybir.dt.float32r
bf16 = mybir.dt.bfloat16
fp8 = mybir.dt.float8e4


def _sview(base, off, dims):
    """Custom (possibly overlapping) strided free-dim view of a 2D SBUF AP."""
    return bass.AP(
        tensor=base.tensor,
        offset=base.offset + off,
        ap=[list(base.ap[0])] + [list(d) for d in dims],
        const_val=base.const_val,
        runtime_checks=base.runtime_checks,
        dep_tracking_offset=base.dep_tracking_offset,
    )


def _emit(nc, tc, ctx, reps=1, sim_compat=False, tune=None, internal_io=False):
    AX = mybir.AxisListType
    OP = mybir.AluOpType
    AF = mybir.ActivationFunctionType
    DR = mybir.MatmulPerfMode.DoubleRow
    tune = {**dict(xbufs=4, obufs=4, rbufs=2, psbufs=3, ybf16=True),
            **(tune or {})}

    io_kind = "Internal" if internal_io else "ExternalInput"
    ydt = bf16 if tune["ybf16"] else f32
    x_d = nc.dram_tensor("x", [BPC, 2, CH, NPIX], bf16, kind=io_kind).ap()
    w1_d = nc.dram_tensor("w1t", [2, CH, 2 * CH], bf16, kind="ExternalInput").ap()
    b1_d = nc.dram_tensor("b1r", [2, CH], f32, kind="ExternalInput").ap()
    wr_d = nc.dram_tensor("wrs", [CH, NE], f32, kind="ExternalInput").ap()
    br_d = nc.dram_tensor("brr", [1, NE], f32, kind="ExternalInput").ap()
    ws_d = nc.dram_tensor("ws8", [CH, NTAP * CH], fp8, kind="ExternalInput").ap()
    bs_d = nc.dram_tensor("bsr", [CH, 1], f32, kind="ExternalInput").ap()
    we_d = nc.dram_tensor("wet", [NE, CH, NTAP * CH], f32, kind="ExternalInput").ap()
    be_d = nc.dram_tensor("ber", [CH, NE], f32, kind="ExternalInput").ap()
    w2_d = nc.dram_tensor("w2t", [3, CH, C2], bf16, kind="ExternalInput").ap()
    b2_d = nc.dram_tensor("b2r", [2, CH], f32, kind="ExternalInput").ap()
    y_d = nc.dram_tensor(
        "y", [BPC, 2, CH, NPIX], ydt,
        kind="Internal" if internal_io else "ExternalOutput").ap()

    wpool = ctx.enter_context(tc.tile_pool(name="weights", bufs=1))
    ppool = ctx.enter_context(tc.tile_pool(name="persist", bufs=1))
    xpool = ctx.enter_context(tc.tile_pool(name="xin", bufs=tune["xbufs"]))
    opool = ctx.enter_context(tc.tile_pool(name="oout", bufs=tune["obufs"]))
    rpool = ctx.enter_context(tc.tile_pool(name="rtile", bufs=tune["rbufs"]))
    spool = ctx.enter_context(tc.tile_pool(name="small", bufs=2))
    selpool = ctx.enter_context(tc.tile_pool(name="sel", bufs=1))
    psum = ctx.enter_context(tc.tile_pool(name="psum", bufs=tune["psbufs"], space="PSUM"))
    psumS = ctx.enter_context(tc.tile_pool(name="psumS", bufs=1, space="PSUM"))

    # ---- load weights into SBUF (resident) ----
    w1_sb = wpool.tile([CH, 2 * 2 * CH], bf16)
    for k in range(2):
        nc.sync.dma_start(w1_sb[:, k * 256:(k + 1) * 256], w1_d[k])
    ws8_sb = wpool.tile([CH, 9 * CH], fp8)
    nc.sync.dma_start(ws8_sb[:], ws_d)
    we_sb = wpool.tile([CH, NE * 9 * CH], f32)
    for e in range(NE):
        nc.sync.dma_start(we_sb[:, e * 1152:(e + 1) * 1152], we_d[e])
    w2_sb = wpool.tile([CH, 3 * C2], bf16)
    for k in range(3):
        nc.sync.dma_start(w2_sb[:, k * 256:(k + 1) * 256], w2_d[k])
    wr_sb = wpool.tile([CH, NE], f32)
    nc.sync.dma_start(wr_sb[:], wr_d)
    br_sb = wpool.tile([1, NE], f32)
    nc.sync.dma_start(br_sb[:], br_d)
    bs_sb = wpool.tile([CH, 1], f32)
    nc.sync.dma_start(bs_sb[:], bs_d)
    be_sb = wpool.tile([CH, NE], f32)
    nc.sync.dma_start(be_sb[:], be_d)
    b1_sb = wpool.tile([CH, 2], f32)
    for k in range(2):
        nc.sync.dma_start(b1_sb[:, k:k + 1], b1_d[k])
    b2_sb = wpool.tile([CH, 2], f32)
    for k in range(2):
        nc.sync.dma_start(b2_sb[:, k:k + 1], b2_d[k])
    ones_sb = wpool.tile([1, CH], f32)
    nc.vector.memset(ones_sb[:], 1.0)

    if internal_io:
        # timing mode: x is Internal (uninitialized) DRAM; zero it once so
        # the timed loop computes on deterministic, non-denormal data.
        zs = wpool.tile([CH, 800], bf16, name="zs")
        nc.vector.memset(zs[:], 0.0)
        for zb in range(BPC):
            for zk in range(2):
                for zj in range(NPIX // 800):
                    nc.sync.dma_start(
                        x_d[zb, zk, :, zj * 800:(zj + 1) * 800], zs[:])

    # ---- persistent per-sample working buffers ----
    # fp (padded feat, bf16) and a are double-buffered across samples so the
    # next sample's cv1 writes don't wait on this sample's cv2 reads.
    fps = []
    for fi in range(2):
        fp = ppool.tile([CH, PADN], bf16, tag=f"fp{fi}", name=f"fp{fi}")
        nc.vector.memset(fp[:], 0.0)   # borders stay zero forever
        fps.append(fp)
    # +4 pad: the discarded (row-wrap) columns of the last conv tile read up
    # to 2 elements past the 82x82 image; keep those reads in-bounds and zero.
    fp8p = ppool.tile([CH, PADN + 4], fp8, name="fp8p")
    nc.vector.memset(fp8p[:].bitcast(f32), 0.0)
    a_sbs = [ppool.tile([CH, NPIX], bf16, tag=f"a{ai}", name=f"a{ai}")
             for ai in range(2)]
    sh_sb = ppool.tile([CH, FLATN], bf16)
    moe_sb = ppool.tile([CH, FLATN], bf16)

    tmpool = ctx.enter_context(tc.tile_pool(name="silutmp", bufs=2)) if sim_compat else None

    def act_silu(out_ap, ps_ap, bias_ap, scale=1.0, accum_ap=None):
        """SiLU from PSUM -> SBUF. On HW, one ACT instruction (with optional
        free GAP accumulation). CoreSim lacks Silu, so sim_compat emulates via
        Sigmoid + (ps*scale+bias)*sig, and computes the accumulation separately."""
        if not sim_compat:
            if accum_ap is not None:
                nc.scalar.activation(out_ap, ps_ap, AF.Silu, bias=bias_ap,
                                     scale=scale, accum_out=accum_ap)
            else:
                nc.scalar.activation(out_ap, ps_ap, AF.Silu, bias=bias_ap,
                                     scale=scale)
            return
        shp = list(out_ap.shape[1:])
        fs = 1
        for d in shp:
            fs *= d
        tmp = tmpool.tile([CH, 2 * 512], f32, tag="sigmoid_tmp")
        tmp2 = tmpool.tile([CH, 2 * 512], f32, tag="scaled_tmp")
        tv = tmp[:, 0:fs]
        sv = tmp2[:, 0:fs]
        if len(shp) == 2:
            tv = tv.rearrange("p (g c) -> p g c", g=shp[0])
            sv = sv.rearrange("p (g c) -> p g c", g=shp[0])
        elif len(shp) == 3:
            tv = tv.rearrange("p (g r c) -> p g r c", g=shp[0], r=shp[1])
            sv = sv.rearrange("p (g r c) -> p g r c", g=shp[0], r=shp[1])
        nc.scalar.activation(tv, ps_ap, AF.Sigmoid, bias=bias_ap, scale=scale)
        # sv = ps*scale + bias ; out = sv * sigmoid(sv)
        nc.vector.tensor_scalar(sv, ps_ap, scale, bias_ap, op0=OP.mult, op1=OP.add)
        nc.vector.tensor_mul(out_ap, sv, tv)
        if accum_ap is not None:
            axis = [None, AX.X, AX.XY, AX.XYZ][len(shp)]
            nc.vector.reduce_sum(accum_ap, out_ap, axis=axis)

    def conv_tile(ps_view, w8, fp8base, t):
        """3x3 conv over flat tile t (410 cols) in fp8: 4 DoubleRow + 1."""
        base = t * RPT * PADW
        for j, (soff, jstr) in enumerate(DR_PAIRS):
            lhs = w8[:, j * 256:(j + 1) * 256].rearrange(
                "p (j c) -> p j c", j=2)
            rhs = _sview(fp8base, base + soff, [[jstr, 2], [1, FN]])
            nc.tensor.matmul(ps_view, lhs, rhs, start=(j == 0), stop=False,
                             perf_mode=DR)
        rhs = _sview(fp8base, base + SINGLE_OFF, [[1, FN]])
        nc.tensor.matmul(ps_view, w8[:, 8 * CH:9 * CH], rhs,
                         start=False, stop=True)

    def _body():
        for b in range(BPC):
            fp = fps[b % 2]
            fp3 = fp[:].rearrange("p (r c) -> p r c", c=PADW)
            a_sb = a_sbs[b % 2]
            fp8base = fp8p[:]
            # ---- cv1 over tile-PAIRS: 800 px per ACT, GAP accumulated free;
            # DVE mirrors feat into the fp8 padded buffer ----
            gap_sb = spool.tile([CH, NP], f32, tag="gap")
            for pi in range(NP):
                i0 = 2 * pi
                xt0 = xpool.tile([CH, 2 * TN], bf16, tag="xt0")
                nc.sync.dma_start(xt0[:], x_d[b, 0, :, pi * 800:(pi + 1) * 800])
                xt1 = xpool.tile([CH, 2 * TN], bf16, tag="xt1")
                nc.sync.dma_start(xt1[:], x_d[b, 1, :, pi * 800:(pi + 1) * 800])
                ps_a = psum.tile([CH, 2, 512], f32, tag="ps")
                ps_f = psum.tile([CH, 2, 512], f32, tag="ps")
                for k, xt in ((0, xt0), (1, xt1)):
                    for hw_, ps2 in ((0, ps_a), (1, ps_f)):
                        wsl = w1_sb[:, k * 256 + hw_ * 128: k * 256 + hw_ * 128 + 128]
                        for ii in range(2):
                            nc.tensor.matmul(ps2[:, ii, 0:TN], wsl,
                                             xt[:, ii * TN:(ii + 1) * TN],
                                             start=(k == 0), stop=(k == 1))
                act_silu(a_sb[:, i0 * TN:(i0 + 2) * TN].rearrange(
                             "p (g c) -> p g c", g=2),
                         ps_a[:, :, 0:TN], b1_sb[:, 0:1])
                fout = fp3[:, 1 + 10 * pi: 11 + 10 * pi, 1:1 + W].rearrange(
                    "p (g r) c -> p g r c", g=2)
                act_silu(fout,
                         ps_f[:, :, 0:TN].rearrange("p g (r c) -> p g r c", c=W),
                         b1_sb[:, 1:2])
                # fp8 mirror of this pair's feat rows (for the 3x3 convs) and
                # the router's GAP partial — both on the (otherwise idle) DVE,
                # keeping the ACT engine free of accum-read overhead.
                nc.vector.tensor_copy(
                    _sview(fp8base, (1 + 10 * pi) * PADW + 1, [[PADW, 10], [1, W]]),
                    fp3[:, 1 + 10 * pi: 11 + 10 * pi, 1:1 + W])
                nc.vector.reduce_sum(gap_sb[:, pi:pi + 1],
                                     fp3[:, 1 + 10 * pi: 11 + 10 * pi, 1:1 + W],
                                     axis=AX.XY)

            # ---- router part 1: pooled sum (wr pre-scaled by 1/NPIX) ----
            pooled = spool.tile([CH, 1], f32, tag="pooled")
            nc.vector.reduce_sum(pooled[:], gap_sb[:], axis=AX.X)

            # ---- shared expert 3x3 conv + SiLU (fp8 DoubleRow), with the
            # router's two tiny PE ops interleaved after tiles 0 and 2 so the
            # PE never stalls on the DVE softmax chain ----
            ps_l = psumS.tile([1, NE], f32, tag="psl")
            ps_bc = psumS.tile([CH, NE + 1], f32, tag="psb")

            def emit_router_mm():
                nc.tensor.matmul(ps_l[:], pooled[:], wr_sb[:], start=True, stop=True)

            def emit_softmax_dve():
                # exp-free softmax: exp(z) = -silu(z)/silu(-z). Keeps the ACT
                # engine in the Silu table (Exp lives in a different table and
                # would force two 1.3us table reloads per sample). The -1e-6
                # shift dodges the 0/0 at the top logit (z == 0).
                logits = spool.tile([1, NE], f32, tag="logits")
                nc.vector.tensor_add(logits[:], ps_l[:], br_sb[:])
                m_sb = spool.tile([1, 1], f32, tag="m")
                nc.vector.reduce_max(m_sb[:], logits[:], axis=AX.X)
                zm = spool.tile([1, NE], f32, tag="zm")
                nc.vector.tensor_scalar(zm[:], logits[:], m_sb[:], 1e-6,
                                        op0=OP.subtract, op1=OP.subtract)
                s1 = spool.tile([1, NE], f32, tag="s1")
                s2 = spool.tile([1, NE], f32, tag="s2")
                if not sim_compat:
                    nc.scalar.activation(s1[:], zm[:], AF.Silu, scale=1.0)
                    nc.scalar.activation(s2[:], zm[:], AF.Silu, scale=-1.0)
                else:
                    sg = spool.tile([1, NE], f32, tag="sg")
                    nc.scalar.activation(sg[:], zm[:], AF.Sigmoid, scale=1.0)
                    nc.vector.tensor_mul(s1[:], zm[:], sg[:])
                    zp = spool.tile([1, NE], f32, tag="zp")
                    nc.vector.tensor_scalar_mul(zp[:], zm[:], -1.0)
                    sg2 = spool.tile([1, NE], f32, tag="sg2")
                    nc.scalar.activation(sg2[:], zp[:], AF.Sigmoid, scale=1.0)
                    nc.vector.tensor_mul(s2[:], zp[:], sg2[:])
                r2 = spool.tile([1, NE], f32, tag="r2")
                nc.vector.reciprocal(r2[:], s2[:])
                e_sb = spool.tile([1, NE], f32, tag="esb")
                nc.vector.scalar_tensor_tensor(e_sb[:], s1[:], -1.0, r2[:],
                                               op0=OP.mult, op1=OP.mult)
                s_sb = spool.tile([1, 1], f32, tag="ssb")
                nc.vector.reduce_sum(s_sb[:], e_sb[:], axis=AX.X)
                wgt = spool.tile([1, 1], f32, tag="wgt")
                nc.vector.reciprocal(wgt[:], s_sb[:])
                oh = spool.tile([1, NE], f32, tag="oh")
                nc.vector.tensor_scalar(oh[:], logits[:], m_sb[:], None, op0=OP.is_ge)
                bc = spool.tile([1, NE + 1], f32, tag="bc")
                nc.vector.tensor_copy(bc[:, 0:NE], oh[:])
                nc.vector.tensor_copy(bc[:, NE:NE + 1], wgt[:])
                return bc

            def emit_bcast_mm(bc):
                nc.tensor.matmul(ps_bc[:], ones_sb[:], bc[:], start=True, stop=True)

            def emit_select():
                # we_sb holds [we0, we1-we0, we2-we0]; with one-hot sc,
                # we0 + sc1*(we1-we0) + sc2*(we2-we0) == we[argmax] exactly
                # (fp8-grid values subtract/add exactly in f32). Two DVE ops,
                # fp8 conversion folded into the second one's output dtype.
                sc = spool.tile([CH, NE + 1], f32, tag="sc")
                nc.vector.tensor_copy(sc[:], ps_bc[:])
                bB = spool.tile([CH, 1], f32, tag="bB")
                nc.vector.scalar_tensor_tensor(bB[:], be_sb[:, 1:2], sc[:, 1:2],
                                               be_sb[:, 0:1], op0=OP.mult, op1=OP.add)
                bS = spool.tile([CH, 1], f32, tag="bS")
                nc.vector.scalar_tensor_tensor(bS[:], be_sb[:, 2:3], sc[:, 2:3],
                                               bB[:], op0=OP.mult, op1=OP.add)
                wB = selpool.tile([CH, 9 * CH], f32, tag="wB")
                nc.vector.scalar_tensor_tensor(wB[:], we_sb[:, 1152:2304], sc[:, 1:2],
                                               we_sb[:, 0:1152], op0=OP.mult, op1=OP.add)
                wS8 = selpool.tile([CH, 9 * CH], fp8, tag="wS8")
                nc.vector.scalar_tensor_tensor(wS8[:], we_sb[:, 2304:3456], sc[:, 2:3],
                                               wB[:], op0=OP.mult, op1=OP.add)
                return wS8, bS, sc

            bc = None
            for pi in range(NP):
                ps2 = psum.tile([CH, 2, 512], f32, tag="ps")
                for ii in range(2):
                    conv_tile(ps2[:, ii, 0:FN], ws8_sb[:], fp8base, 2 * pi + ii)
                act_silu(sh_sb[:, pi * 2 * FN:(pi + 1) * 2 * FN].rearrange(
                             "p (g c) -> p g c", g=2),
                         ps2[:, :, 0:FN], bs_sb[:], scale=1.0 / WSCALE)
                if pi == 0:
                    emit_router_mm()
                    bc = emit_softmax_dve()
                elif pi == 2:
                    emit_bcast_mm(bc)
                elif pi == 3:
                    wS8, bS, sc = emit_select()

            # ---- routed conv (fp8 DR) + moe + fused cv2 (bf16),
            # software-pipelined by one pair ----
            def cv2_pair(pi):
                i0 = 2 * pi
                moe3 = moe_sb[:].rearrange("p (r c) -> p r c", c=PADW)
                for h in range(2):
                    po = psum.tile([CH, 2, 512], f32, tag="ps")
                    for ii in range(2):
                        i = i0 + ii
                        ft = fp3[:, i * RPT + 1: i * RPT + 1 + RPT, 1: 1 + W]
                        mt = moe3[:, i * RPT: i * RPT + RPT, 0:W]
                        nc.tensor.matmul(po[:, ii, 0:TN],
                                         w2_sb[:, h * 128: h * 128 + 128],
                                         a_sb[:, i * TN:(i + 1) * TN],
                                         start=True, stop=False)
                        nc.tensor.matmul(po[:, ii, 0:TN],
                                         w2_sb[:, 256 + h * 128: 256 + h * 128 + 128],
                                         ft, start=False, stop=False)
                        nc.tensor.matmul(po[:, ii, 0:TN],
                                         w2_sb[:, 512 + h * 128: 512 + h * 128 + 128],
                                         mt, start=False, stop=True)
                    ot = opool.tile([CH, 2 * TN], ydt, tag="ot")
                    act_silu(ot[:].rearrange("p (g c) -> p g c", g=2),
                             po[:, :, 0:TN], b2_sb[:, h:h + 1])
                    # y stores ride the ACT DMA queue (keeps SP free for x)
                    nc.scalar.dma_start(y_d[b, h, :, pi * 800:(pi + 1) * 800], ot[:])

            for pi in range(NP):
                ps2 = psum.tile([CH, 2, 512], f32, tag="ps")
                for ii in range(2):
                    conv_tile(ps2[:, ii, 0:FN], wS8[:], fp8base, 2 * pi + ii)
                rt = rpool.tile([CH, 2 * FN], bf16, tag="rt")
                act_silu(rt[:].rearrange("p (g c) -> p g c", g=2),
                         ps2[:, :, 0:FN], bS[:], scale=1.0 / WSCALE)
                nc.vector.scalar_tensor_tensor(
                    moe_sb[:, pi * 2 * FN:(pi + 1) * 2 * FN], rt[:], sc[:, NE:NE + 1],
                    sh_sb[:, pi * 2 * FN:(pi + 1) * 2 * FN], op0=OP.mult, op1=OP.add)
                if pi > 0:
                    cv2_pair(pi - 1)
            cv2_pair(NP - 1)

    if reps == 1:
        _body()
    else:
        # HW timing mode: repeat the whole workload in a hardware loop
        # (same instruction count / compile cost; R x device work).
        with tc.For_i(0, reps, 1):
            _body()
    if internal_io:
        # tiny external output so the (otherwise internal-IO) program is not
        # dead-code eliminated; depends on the looped work via y.
        ydig_d = nc.dram_tensor("ydig", [CH, 4], ydt,
                                kind="ExternalOutput").ap()
        ydig_t = opool.tile([CH, 4], ydt, name="ydig_t")
        nc.sync.dma_start(ydig_t[:], y_d[0, 0, :, 0:4])
        nc.sync.dma_start(ydig_d, ydig_t[:])


def build(reps=1, sim_compat=False, tune=None, internal_io=False):
    from contextlib import ExitStack
    nc = bacc.Bacc("TRN2", target_bir_lowering=False, debug=False,
                   num_devices=NCORES)
    with tile.TileContext(nc) as tc:
        with ExitStack() as ctx:
            _emit(nc, tc, ctx, reps=reps, sim_compat=sim_compat, tune=tune,
                  internal_io=internal_io)
    nc.compile()
    return nc


def marshal_inputs(x, w1, b1, wr, br, ws, bs, we, be, w2, b2, ybf16=True):
    """Host-side (tiny) weight re-layouts into matmul-friendly forms."""
    import ml_dtypes
    asf = lambda a: np.ascontiguousarray(np.asarray(a, dtype=np.float32))
    cvtb = lambda a: np.ascontiguousarray(
        np.asarray(a, np.float32).astype(ml_dtypes.bfloat16))
    cvt8 = lambda a: np.ascontiguousarray(
        np.asarray(a, np.float32).astype(ml_dtypes.float8_e4m3))
    x = cvtb(x)
    w1t = cvtb(np.asarray(w1, np.float32).reshape(2 * CH, C1).T.reshape(2, CH, 2 * CH))
    b1r = asf(np.asarray(b1, np.float32).reshape(2, CH))
    wrs = asf(np.asarray(wr, np.float32) / NPIX)
    brr = asf(np.asarray(br, np.float32).reshape(1, NE))
    # conv weights: [cin, tap, cout] with taps permuted for DoubleRow pairing,
    # pre-scaled by 2^8 into e4m3's normal range.
    wst = np.asarray(ws, np.float32).transpose(1, 2, 3, 0).reshape(CH, 9, CH)
    ws8 = cvt8(wst[:, TAP_PERM, :].reshape(CH, 9 * CH) * WSCALE)
    bsr = asf(np.asarray(bs, np.float32).reshape(CH, 1))
    wet_ = np.asarray(we, np.float32).transpose(0, 2, 3, 4, 1).reshape(NE, CH, 9, CH)
    # quantize to the fp8 grid on host, ship as f32 in difference form
    # [we0, we1-we0, we2-we0] (exact 2-op one-hot select on device)
    wet = asf(cvt8(wet_[:, :, TAP_PERM, :].reshape(NE, CH, 9 * CH) * WSCALE))
    wet[1] -= wet[0]
    wet[2] -= wet[0]
    ber = asf(np.asarray(be, np.float32).T)
    ber[:, 1] -= ber[:, 0]
    ber[:, 2] -= ber[:, 0]
    w2t = cvtb(np.asarray(w2, np.float32).reshape(C2, 3 * CH).T.reshape(3, CH, C2))
    b2r = asf(np.asarray(b2, np.float32).reshape(2, CH))
    shared = dict(w1t=w1t, b1r=b1r, wrs=wrs, brr=brr, ws8=ws8, bsr=bsr,
                  wet=wet, ber=ber, w2t=w2t, b2r=b2r)
    xc = x.reshape(NCORES, BPC, 2, CH, NPIX)
    in_maps = [dict(shared, x=np.ascontiguousarray(xc[c])) for c in range(NCORES)]
    return in_maps


_CACHE = {}


def _get_nc():
    if "nc" not in _CACHE:
        _CACHE["nc"] = build(reps=1)
    return _CACHE["nc"]


def _get_runner():
    """Build the sharded PJRT callable once (mirrors
    bass2jax.run_bass_via_pjrt's multi-core path) so repeat kernel() calls
    skip the jax retrace/compile."""
    if "runner" in _CACHE:
        return _CACHE["runner"]
    import jax
    from jax.experimental.shard_map import shard_map
    from jax.sharding import Mesh, PartitionSpec
    from concourse import bass2jax

    nc = _get_nc()
    bass2jax.install_neuronx_cc_hook()
    part_name = nc.partition_id_tensor.name if nc.partition_id_tensor else None
    in_names, out_names, out_avals = [], [], []
    for alloc in nc.m.functions[0].allocations:
        if not isinstance(alloc, mybir.MemoryLocationSet):
            continue
        name = alloc.memorylocations[0].name
        if alloc.kind == "ExternalInput":
            if name != part_name:
                in_names.append(name)
        elif alloc.kind == "ExternalOutput":
            out_names.append(name)
            out_avals.append(jax.core.ShapedArray(
                tuple(alloc.tensor_shape), mybir.dt.np(alloc.dtype)))
    assert nc.dbg_addr is None
    n_params = len(in_names)
    all_in = in_names + out_names  # zero buffers donated as outputs
    if part_name is not None:
        all_in = all_in + [part_name]

    def _body(*args):
        operands = list(args)
        if part_name is not None:
            operands.append(bass2jax.partition_id_tensor())
        outs = bass2jax._bass_exec_p.bind(
            *operands, out_avals=tuple(out_avals), in_names=tuple(all_in),
            out_names=tuple(out_names), lowering_input_output_aliases=(),
            sim_require_finite=True, sim_require_nnan=True, nc=nc)
        return tuple(outs)

    devices = jax.devices()[:NCORES]
    mesh = Mesh(np.asarray(devices), ("core",))
    nio = n_params + len(out_names)
    sharded = jax.jit(
        shard_map(_body, mesh=mesh, in_specs=(PartitionSpec("core"),) * nio,
                  out_specs=(PartitionSpec("core"),) * len(out_names),
                  check_rep=False),
        donate_argnums=tuple(range(n_params, nio)), keep_unused=True)
    _CACHE["runner"] = (sharded, in_names, out_names, out_avals)
    return _CACHE["runner"]


def kernel(x, w1, b1, wr, br, ws, bs, we, be, w2, b2):
    in_maps = marshal_inputs(x, w1, b1, wr, br, ws, bs, we, be, w2, b2)
    sharded, in_names, out_names, out_avals = _get_runner()
    concat_in = [
        np.concatenate([in_maps[c][name] for c in range(NCORES)], axis=0)
        for name in in_names
    ]
    concat_zeros = [
        np.zeros((NCORES * a.shape[0], *a.shape[1:]), a.dtype) for a in out_avals
    ]
    out_arrs = sharded(*concat_in, *concat_zeros)
    y = np.asarray(out_arrs[out_names.index("y")]).astype(np.float32)
    return np.ascontiguousarray(y.reshape(B, C2, H, W))
